# revision 86
# baseline (speedup 1.0000x reference)
"""Multi-head attention (B=2,T=2048,C=1024,H=16,RoPE,causal) on 8 TRN2 cores.

Sharding: core c -> (batch b = c//4, head-group g = c%4, heads [4g,4g+4)).
Each core computes QKV projection for its 4 heads against x[b], RoPE,
causal attention, and the output projection rows t' in [512g, 512g+512)
of y[b] (the reference's (B,H,T,Dh)->(B,T,C) reshape makes output blocks
head-disjoint).

Schedule: software-pipelined over u = (hp, tt) slots.  Slot s emits
finalize(u=s-2), then weaves QKV/RoPE work A(s) between attention chunks
B(s-1) so the PE never drains (p-state) and exp latency is hidden.
Attention is computed in the transposed [s, t] layout; AV keeps V
stationary ([s,Dh+1], ones column = softmax denominator) and streams
only the causally-needed probability columns.  Normalization broadcasts
1/Z across partitions with a tiny ones-matmul on the PE, and the final
multiply scatters directly into the bit-permuted attP layout
(attP[d, j*128+k] = att[d, t=16k+j]) so the output projection's
head-reshape gather becomes contiguous copies.  Engine placement by
measured HW costs: exps + psum->sbuf q/k casts on Scalar, RoPE muls and
psum copies on Vector, masks + aht copies on GpSimd (SBUF-only there).
"""
import math
import sys

sys.path.insert(0, '/opt/trn_rl_repo')
sys.path.insert(0, '/opt/pypackages')

import ml_dtypes
import numpy as np
from contextlib import ExitStack

import concourse.bass as bass  # noqa: F401
import concourse.tile as tile
from concourse import bacc, mybir
from concourse.bass_utils import run_bass_kernel_spmd

BF16 = mybir.dt.bfloat16
F32 = mybir.dt.float32
NPBF16 = ml_dtypes.bfloat16

B, T, C, H, Dh = 2, 2048, 1024, 16, 64
HALF = Dh // 2          # 32
NCORES = 8
HPC = 4                 # heads per core
CPC = HPC * Dh          # channels per core = 256
SCALE = 1.0 / math.sqrt(Dh)
TT = 512                # t-tile width
NTT = T // TT           # 4
SC = 128                # s-chunk width
NU = 2 * NTT            # pipeline slots: (hp, tt)

_compiled_nc = None


def _build_nc(dbg=False):
    nc = bacc.Bacc("TRN2", target_bir_lowering=False, debug=False)

    xT = nc.dram_tensor("xT", [C, T], BF16, kind="ExternalInput").ap()
    wqkvT = nc.dram_tensor("wqkvT", [C, 3 * CPC], BF16, kind="ExternalInput").ap()
    wpT = nc.dram_tensor("wpT", [C, C], BF16, kind="ExternalInput").ap()
    cosx = nc.dram_tensor("cosx", [128, T], BF16, kind="ExternalInput").ap()
    sinx = nc.dram_tensor("sinx", [128, T], BF16, kind="ExternalInput").ap()
    rt = nc.dram_tensor("rt", [128, 128], BF16, kind="ExternalInput").ap()
    ident = nc.dram_tensor("ident", [128, 128], BF16, kind="ExternalInput").ap()
    mask01 = nc.dram_tensor("mask01", [128, 2, 128], BF16,
                            kind="ExternalInput").ap()
    yblk = nc.dram_tensor("yblk", [512, C], BF16, kind="ExternalOutput").ap()
    if dbg:
        qdbg = nc.dram_tensor("qdbg", [128, T], BF16, kind="ExternalOutput").ap()
        kdbg = nc.dram_tensor("kdbg", [128, T], BF16, kind="ExternalOutput").ap()
        vdbg = nc.dram_tensor("vdbg", [128, T // SC, 132], BF16,
                              kind="ExternalOutput").ap()
        adbg = nc.dram_tensor("adbg", [128, T], BF16,
                              kind="ExternalOutput").ap()

    with tile.TileContext(nc) as tc, ExitStack() as ctx:
        const = ctx.enter_context(tc.tile_pool(name="const", bufs=1))
        qkpool = ctx.enter_context(tc.tile_pool(name="qk", bufs=2))
        vpool = ctx.enter_context(tc.tile_pool(name="vnat", bufs=2))
        gbpool = ctx.enter_context(tc.tile_pool(name="gbp", bufs=2))
        tmp = ctx.enter_context(tc.tile_pool(name="tmp", bufs=2))
        pbpool = ctx.enter_context(tc.tile_pool(name="pbp", bufs=6))
        attp = ctx.enter_context(tc.tile_pool(name="attp", bufs=2))
        anat = ctx.enter_context(tc.tile_pool(name="anat", bufs=10))
        yopool = ctx.enter_context(tc.tile_pool(name="yop", bufs=4))
        zipool = ctx.enter_context(tc.tile_pool(name="zip", bufs=10))
        psA = ctx.enter_context(tc.tile_pool(name="psA", bufs=2, space="PSUM"))
        psQK = ctx.enter_context(tc.tile_pool(name="psQK", bufs=2, space="PSUM"))
        psB = ctx.enter_context(tc.tile_pool(name="psB", bufs=1, space="PSUM"))

        # ---- constants (in need-order so early compute is unblocked) ----
        wqkv_sb = const.tile([128, 8, 3 * CPC], BF16)
        wqkv_src = wqkvT.rearrange("(cc p) f -> p cc f", p=128)
        x_sb = []
        for tt in range(NTT):
            xs = const.tile([128, 8, TT], BF16, name=f"x_sb{tt}")
            x_sb.append(xs)
        x_src = xT.rearrange("(cc p) t -> p cc t", p=128)
        nc.sync.dma_start(x_sb[0][:, 0:4, :], x_src[:, 0:4, 0:TT])
        nc.sync.dma_start(x_sb[0][:, 4:8, :], x_src[:, 4:8, 0:TT])
        # hp0's q/k/v weight columns first so slot 0 can start early
        for gi in range(3):
            f0 = gi * CPC
            nc.sync.dma_start(wqkv_sb[:, :, f0:f0 + 128],
                              wqkv_src[:, :, f0:f0 + 128])
        rt_sb = const.tile([128, 128], BF16)
        nc.sync.dma_start(rt_sb[:], rt[:])
        cos_sb = const.tile([128, T], BF16)
        sin_sb = const.tile([128, T], BF16)
        nc.sync.dma_start(cos_sb[:, 0:TT], cosx[:, 0:TT])
        nc.sync.dma_start(sin_sb[:, 0:TT], sinx[:, 0:TT])
        mask_sb = const.tile([128, 2, 128], BF16)
        nc.sync.dma_start(mask_sb[:], mask01[:])
        id_sb = const.tile([128, 128], BF16)
        nc.sync.dma_start(id_sb[:], ident[:])
        for gi in range(3):
            f0 = gi * CPC + 128
            nc.sync.dma_start(wqkv_sb[:, :, f0:f0 + 128],
                              wqkv_src[:, :, f0:f0 + 128])
        for tt in range(1, NTT):
            nc.sync.dma_start(x_sb[tt][:], x_src[:, :, tt * TT:(tt + 1) * TT])
            nc.sync.dma_start(cos_sb[:, tt * TT:(tt + 1) * TT],
                              cosx[:, tt * TT:(tt + 1) * TT])
            nc.sync.dma_start(sin_sb[:, tt * TT:(tt + 1) * TT],
                              sinx[:, tt * TT:(tt + 1) * TT])
        wt_sb = const.tile([128, 8, C], BF16)

        # per-hp persistent state
        state = {}

        def alloc_hp(hp):
            q_sb = qkpool.tile([128, T], BF16, tag="q", name=f"q_sb{hp}")
            k_sb = qkpool.tile([128, T], BF16, tag="k", name=f"k_sb{hp}")
            # both heads in one tile: hl block at 66*hl, data [0:64], one@64
            v_nat = vpool.tile([128, T // SC, 132], BF16, tag="v",
                               name=f"vnat_{hp}")
            vv = v_nat.rearrange("p j (h c) -> p j h c", h=2)
            nc.vector.memset(vv[:, :, :, 64:65], 1.0)
            # attP2[hl][64*(j%2)+d, (j//2)*128+k] = att_hl[d, t=16k+j]:
            # the output projection reads 128-col slices of this directly
            # as its stationary operand (no gather copies needed).
            attP = [attp.tile([128, T // 2], BF16, tag=f"attP{hl}",
                              name=f"attP_{hp}_{hl}") for hl in range(2)]
            state[hp] = dict(q=q_sb, k=k_sb, v=v_nat, attP=attP)

        # ---------------- A(u): QKV projection + RoPE + v transpose -------
        def a_thunks(u):
            hp, tt = u // 4, u % 4
            if tt == 0:
                alloc_hp(hp)
            st = state[hp]
            ts = slice(tt * TT, (tt + 1) * TT)
            thunks = []

            def mk_qk(grp, gi, gps_ref, gb_ref):
                # two half-thunks for finer weave granularity
                def th_a():
                    f0 = gi * CPC + hp * 128
                    gps = psA.tile([128, TT], F32, tag="mm",
                                   name=f"gps_{u}_{gi}")
                    for cc in range(4):
                        nc.tensor.matmul(
                            gps[:], wqkv_sb[:, cc, f0:f0 + 128],
                            x_sb[tt][:, cc, :],
                            start=(cc == 0), stop=False,
                            skip_group_check=True)
                    gps_ref[0] = gps

                def th_b():
                    f0 = gi * CPC + hp * 128
                    gps = gps_ref[0]
                    for cc in range(4, 8):
                        nc.tensor.matmul(
                            gps[:], wqkv_sb[:, cc, f0:f0 + 128],
                            x_sb[tt][:, cc, :],
                            start=False, stop=(cc == 7),
                            skip_group_check=True)
                    gb = gbpool.tile([128, TT], BF16, tag=grp,
                                     name=f"gb_{u}_{grp}")
                    nc.vector.tensor_copy(gb[:], gps[:])
                    gb_ref[0] = gb
                return th_a, th_b

            def mk_rot(grp, gb_ref):
                def th():
                    gb = gb_ref[0]
                    rot_ps = psA.tile([128, TT], F32, tag="mm",
                                      name=f"rot_{u}_{grp}")
                    nc.tensor.matmul(rot_ps[:], rt_sb[:], gb[:],
                                     start=True, stop=True)
                    gc = tmp.tile([128, TT], F32, tag="gc" + grp,
                                  name=f"gc_{u}_{grp}")
                    nc.vector.tensor_mul(gc[:], gb[:], cos_sb[:, ts])
                    gs = tmp.tile([128, TT], F32, tag="gs" + grp,
                                  name=f"gs_{u}_{grp}")
                    nc.vector.tensor_mul(gs[:], rot_ps[:], sin_sb[:, ts])
                    dest = st["q"] if grp == "q" else st["k"]
                    nc.vector.tensor_add(dest[:, ts], gc[:], gs[:])
                return th

            # B(u-1) with few chunks leaves the Scalar engine idle: route
            # the psV drain copies there so the Vector queue frees psA
            # faster; in exp-heavy slots keep them on Vector.
            prev_njs = 4 * ((u - 1) % 4) + 4 if u >= 1 else 0
            vt_on_act = prev_njs <= 8

            def mk_vt(tc):
                # transposed V projection: out[t, f] directly (stationary
                # x chunk, moving w_v slice) -> no transpose needed
                def th():
                    ci = tt * 4 + tc
                    f0 = 2 * CPC + hp * 128
                    psV = psA.tile([128, 128], F32, tag="mm",
                                   name=f"psV_{u}_{tc}")
                    for cc in range(8):
                        nc.tensor.matmul(
                            psV[:], x_sb[tt][:, cc, tc * 128:(tc + 1) * 128],
                            wqkv_sb[:, cc, f0:f0 + 128],
                            start=(cc == 0), stop=(cc == 7))
                    dst = st["v"][:, ci, 0:132].rearrange(
                        "p (h c) -> p h c", h=2)[:, :, 0:64]
                    src = psV[:].rearrange("p (h c) -> p h c", h=2)
                    if vt_on_act:
                        nc.scalar.copy(dst, src)
                    else:
                        nc.vector.tensor_copy(dst, src)
                return th

            gbq_ref, gbk_ref = [None], [None]
            gpsq_ref, gpsk_ref = [None], [None]

            qa, qb = mk_qk("q", 0, gpsq_ref, gbq_ref)
            ka, kb = mk_qk("k", 1, gpsk_ref, gbk_ref)
            thunks.append(qa)
            thunks.append(qb)
            thunks.append(ka)
            thunks.append(kb)
            thunks.append(mk_rot("q", gbq_ref))
            thunks.append(mk_vt(0))
            thunks.append(mk_vt(1))
            thunks.append(mk_rot("k", gbk_ref))
            thunks.append(mk_vt(2))
            thunks.append(mk_vt(3))
            return thunks

        # ---------------- B(u): causal attention chunks -------------------
        psat_state = {}

        def b_thunks(u):
            hp, tt = u // 4, u % 4
            st = state[hp]
            njs = 4 * tt + 4
            psb = [psB.tile([128, 4, Dh + 1], F32, tag=f"b{hl}",
                            name=f"psb_{u}_{hl}") for hl in range(2)]
            # start_tensor_calc pending-zeroes the whole 2KB bank, which
            # would wipe sibling tc-groups: zero once, accumulate always.
            for hl in range(2):
                nc.vector.memset(psb[hl][:], 0.0)
            psat_state[u] = psb
            pb_tiles = {}

            def mk_scores(j):
                def th():
                    off = max(0, 128 * (j - 4 * tt))
                    sj = slice(j * SC, (j + 1) * SC)
                    qk = psQK.tile([128, 2, TT], F32, tag="qk",
                                   name=f"qk_{u}_{j}")
                    for hl in range(2):
                        hb = hl * 64
                        nc.tensor.matmul(
                            qk[:, hl, off:TT],
                            st["k"][hb:hb + 64, sj],
                            st["q"][hb:hb + 64, tt * TT + off:(tt + 1) * TT],
                            start=True, stop=True)
                    pb = pbpool.tile([128, 2, TT], BF16, tag="pb",
                                     name=f"pb_{u}_{j}")
                    nc.scalar.activation(
                        pb[:, :, off:], qk[:, :, off:],
                        mybir.ActivationFunctionType.Exp, scale=SCALE)
                    if j >= 4 * tt:
                        for hl in range(2):
                            dsl = pb[:, hl, off:off + 128]
                            nc.vector.tensor_mul(dsl, dsl, mask_sb[:, hl, :])
                    pb_tiles[j] = pb
                return th

            def mk_av(j):
                def th():
                    off = max(0, 128 * (j - 4 * tt))
                    pb = pb_tiles[j]
                    for tc in range(off // 128, 4):
                        for hl in range(2):
                            nc.tensor.matmul(
                                psb[hl][:, tc, :],
                                pb[:, hl, tc * 128:(tc + 1) * 128],
                                st["v"][:, j, 66 * hl:66 * hl + 65],
                                start=False, stop=(j == 4 * tt + tc),
                                skip_group_check=True)
                return th

            # AV trails scores by 4 chunks so the PE queue never blocks on
            # an exp (or on the F-drain + memset of the psb banks).
            LAG = 4
            thunks = []
            for j in range(njs + LAG):
                def mk_pair(j):
                    s_th = mk_scores(j) if j < njs else None
                    a_th = mk_av(j - LAG) if j >= LAG else None
                    def th():
                        if s_th:
                            s_th()
                        if a_th:
                            a_th()
                    return th
                thunks.append(mk_pair(j))
            return thunks

        # ---------------- F(u): normalize into parity-split attP ----------
        # f_drain: per (hl, tc) reciprocal + per-partition normalize (frees
        # the psb banks; cheap — Z is a per-partition scalar in [t, d]
        # layout) + PE transpose back to [d, t].  The attP scatter copies
        # feed only the projection and run as latency-tolerant GpSimd
        # bursts (GpSimd op dispatch has us-scale latency, so nothing
        # may ever wait on it in-slot).
        f_state = {}

        def f_drain(u, tail=False):
            psb = psat_state.pop(u)
            nats = {}
            for hl in range(2):
                for tcp in range(2):
                    # two adjacent t-chunks share one tile so one PE
                    # transpose handles both
                    nat2 = anat.tile([128, 2, Dh], BF16, tag="nat",
                                     name=f"nat_{u}_{hl}_{tcp}")
                    for h2 in range(2):
                        tc = 2 * tcp + h2
                        zi = zipool.tile([128, 1], F32, tag="zi",
                                         name=f"zi_{u}_{hl}_{tc}")
                        nc.vector.reciprocal_approx_fast(
                            out=zi[:], in_=psb[hl][:, tc, Dh:Dh + 1])
                        if tail and hl == 1:
                            # idle Scalar engine halves the tail chain
                            nc.scalar.activation(
                                nat2[:, h2, :], psb[hl][:, tc, 0:Dh],
                                mybir.ActivationFunctionType.Copy,
                                scale=zi[:])
                        else:
                            nc.vector.tensor_scalar_mul(
                                nat2[:, h2, :], psb[hl][:, tc, 0:Dh], zi[:])
                    nats[(hl, tcp)] = nat2
            f_state[u] = nats

        def f_norm_thunks(u, scatter_eng=None):
            hp, tt = u // 4, u % 4
            st = state[hp]
            nats = f_state.pop(u)
            thunks = []
            for hl in range(2):
                for tcp in range(2):
                    def th(hl=hl, tcp=tcp):
                        tps = psA.tile([128, 128], BF16, tag="mm",
                                       name=f"ftps_{u}_{hl}_{tcp}")
                        nat2 = nats[(hl, tcp)]
                        nc.tensor.transpose(
                            tps[:], nat2.rearrange("p h d -> p (h d)"),
                            id_sb[:])
                        tsb = anat.tile([128, 128], BF16, tag="tsb",
                                        name=f"tsb_{u}_{hl}_{tcp}", bufs=16)
                        if scatter_eng is nc.vector and hl == 1:
                            nc.scalar.copy(tsb[:], tps[:])
                        else:
                            nc.vector.tensor_copy(tsb[:], tps[:])
                        # tsb row 64*h2+d, col i (t = 128g + i, g=4tt+2tcp+h2)
                        # i = 16*kk + 2*j2 + par ->
                        # attP[hl][64*par + d, j2*128 + 8*g + kk]
                        for h2 in range(2):
                            g = 4 * tt + 2 * tcp + h2
                            src4 = tsb[64 * h2:64 * h2 + 64, :].rearrange(
                                "d (k j2 p2) -> d p2 j2 k", j2=8, p2=2)
                            for par in range(2):
                                dst = st["attP"][hl][
                                    64 * par:64 * par + 64, :].rearrange(
                                    "d (j2 k) -> d j2 k",
                                    j2=8)[:, :, 8 * g:8 * g + 8]
                                if scatter_eng is nc.vector and hl == 1:
                                    nc.scalar.copy(dst, src4[:, par])
                                else:
                                    (scatter_eng or nc.gpsimd).tensor_copy(
                                        dst, src4[:, par])
                    thunks.append(th)
            return thunks

        # ---------------- output projection (reads attP directly) ---------
        def proj_thunks(hp):
            thunks = []
            for hl in range(2):
                for ot in range(2):
                    def th(hl=hl, ot=ot):
                        st = state[hp]
                        r0 = (hp * 2 + hl) * 128
                        yps = psA.tile([128, 512], F32, tag="mm",
                                       name=f"yps_{hp}_{hl}_{ot}")
                        for cc in range(8):
                            nc.tensor.matmul(
                                yps[:],
                                st["attP"][hl][:, cc * 128:(cc + 1) * 128],
                                wt_sb[:, cc, ot * 512:(ot + 1) * 512],
                                start=(cc == 0), stop=(cc == 7))
                        yo = yopool.tile([128, 512], BF16, tag="yo",
                                         name=f"yo_{hp}_{hl}_{ot}")
                        nc.vector.tensor_copy(yo[:], yps[:])
                        nc.sync.dma_start(
                            yblk[r0:r0 + 128, ot * 512:(ot + 1) * 512],
                            yo[:])
                    thunks.append(th)
            return thunks

        # ---------------- weave + slot loop -------------------------------
        def weave(primary, filler):
            if not primary:
                for f in filler:
                    f()
                return
            n, m = len(primary), len(filler)
            fi = 0
            for i, p in enumerate(primary):
                p()
                want = (i + 1) * m // n
                while fi < want:
                    filler[fi]()
                    fi += 1
            while fi < m:
                filler[fi]()
                fi += 1

        for s in range(NU + 1):
            if s == 4:
                nc.sync.dma_start(
                    wt_sb[:], wpT.rearrange("(cc p) o -> p cc o", p=128))
            fnorm = []
            if s >= 2:
                f_drain(s - 2)
                fnorm = f_norm_thunks(s - 2)
            if s == 7:
                p0 = proj_thunks(0)
                filler = fnorm + a_thunks(s) + [p0[0]]
            elif s == NU:
                filler = fnorm + p0[1:]
            else:
                filler = fnorm + a_thunks(s)
            primary = b_thunks(s - 1) if s >= 1 else []
            weave(primary, filler)
        # tail: scatters on DVE/ACT (latency-bound here; nothing else to do);
        # each head-half's projection starts as soon as its scatters land
        f_drain(NU - 1, tail=True)
        fn7 = f_norm_thunks(NU - 1, scatter_eng=nc.vector)
        p1 = proj_thunks(1)
        for th in (fn7[0], fn7[1], p1[0], fn7[2], p1[1],
                   fn7[3], p1[2], p1[3]):
            th()
        if dbg:
            nc.sync.dma_start(qdbg[:], state[0]["q"][:])
            nc.sync.dma_start(kdbg[:], state[0]["k"][:])
            nc.sync.dma_start(vdbg[:], state[0]["v"][:])
            nc.sync.dma_start(adbg[:, 0:T // 2], state[0]["attP"][0][:])
            nc.sync.dma_start(adbg[:, T // 2:T], state[0]["attP"][1][:])

    nc.compile()
    return nc


def _get_nc():
    global _compiled_nc
    if _compiled_nc is None:
        _compiled_nc = _build_nc()
    return _compiled_nc


def _host_tables():
    pos = np.arange(T, dtype=np.float32)[:, None]
    inv = np.exp(np.arange(0, Dh, 2, dtype=np.float32)
                 * (-math.log(10000.0) / Dh))
    ang = pos * inv                       # (T, 32)
    sin, cos = np.sin(ang), np.cos(ang)   # (T, 32)
    idx = np.arange(128) % HALF           # d % 32
    cos_ext = cos[:, idx].T.astype(NPBF16)  # (128, T)
    sin_ext = sin[:, idx].T.astype(NPBF16)

    R = np.zeros((128, 128), dtype=np.float32)
    for blk in (0, 64):
        for m in range(HALF):
            R[blk + m, blk + m + HALF] = -1.0
            R[blk + m + HALF, blk + m] = 1.0
    rt = np.ascontiguousarray(R.T).astype(NPBF16)

    s_i = np.arange(128)[:, None]
    t_i = np.arange(128)[None, :]
    mask01 = (t_i >= s_i).astype(np.float32).astype(NPBF16)
    mask01 = np.ascontiguousarray(
        np.broadcast_to(mask01[:, None, :], (128, 2, 128)))
    ident = np.eye(128, dtype=np.float32).astype(NPBF16)
    return cos_ext, sin_ext, rt, mask01, ident


def kernel(x, w_qkv, w_proj):
    x = np.asarray(x)
    w_qkv = np.asarray(w_qkv)
    w_proj = np.asarray(w_proj)
    nc = _get_nc()
    in_maps = build_in_maps(x, w_qkv, w_proj)
    res = run_bass_kernel_spmd(nc, in_maps, core_ids=list(range(NCORES)))
    y = np.zeros((B, T, C), dtype=np.float32)
    for c in range(NCORES):
        b, g = c // 4, c % 4
        y[b, 512 * g:512 * g + 512, :] = \
            res.results[c]["yblk"].astype(np.float32)
    return y


def build_in_maps(x, w_qkv, w_proj):
    cos_ext, sin_ext, rt, mask01, ident = _host_tables()
    wq4 = w_qkv.reshape(3, H, Dh, C)
    wpT = np.ascontiguousarray(w_proj.T.astype(NPBF16))
    in_maps = []
    for c in range(NCORES):
        b, g = c // 4, c % 4
        hs = slice(4 * g, 4 * g + 4)
        wq = wq4[0, hs].reshape(CPC, C)
        wk = wq4[1, hs].reshape(CPC, C)
        wv = wq4[2, hs].reshape(CPC, C)
        wqkvT = np.concatenate([wq, wk, wv], axis=0).T.astype(NPBF16)
        xT = x[b].T.astype(NPBF16)
        in_maps.append({
            "xT": np.ascontiguousarray(xT),
            "wqkvT": np.ascontiguousarray(wqkvT),
            "wpT": wpT,
            "cosx": cos_ext, "sinx": sin_ext,
            "rt": rt, "ident": ident, "mask01": mask01,
        })
    return in_maps
